# revision 35
# baseline (speedup 1.0000x reference)
"""Bahdanau additive attention kernel for 8 Trainium2 NeuronCores.

Math (per batch element b):
    pq = query[b] @ Wq.T                       [Q, NU]
    pk = keys[b]  @ Wk.T                       [K, NU]
    v  = linear_att / ||linear_att|| * normalize_scalar
    scores[q,k] = sum_u tanh(pq[q,u] + pk[k,u] + bias[u]) * v[u]
    scores_normalized = softmax(scores, -1)
    context = scores @ keys[b]                 (un-normalized scores, faithful)

Sharding: data parallel over batch, B == 8 == n_cores, no collectives.

Per-core pipeline:
    PE   : pqT[u,q], pkT[u,k] projections (float32r matmuls)
    DVE  : S[u, (q,k)-chunk] = pkT + pq[q]   (tensor_scalar add, 2x mode)
    ACT  : T = tanh(S) in large-free-dim instructions, output fp16
    PE   : scoresT[k,q] = sum_u T[u,k] * v[u]  (fp16 matvec, PSUM accum)
    per q-block tail: transpose + softmax + context (overlaps next block)
"""

import sys

for _p in ("/opt/trn_rl_repo",):
    if _p not in sys.path:
        sys.path.insert(0, _p)

import numpy as np

B, Q, K, D, NU = 8, 64, 512, 512, 512
UT = NU // 128  # u tiles
KT = K // 128   # k tiles
DT = D // 128   # d tiles
QH = 32         # q's per tail half
# variable hot-loop chunk sizes per half: small at head (fast ACT ramp) and
# at the very end (small final matvec burst before the tail chain)
CHUNKS = [[2, 4, 8, 8, 10], [10, 10, 8, 2, 2]]
QBMAX = 10
N_CORES = 8
WDT16 = True    # fp16 weights/keys for projection + context matmuls

_CACHE = {}


def _build(variant="full", repeat=1, wdt16=WDT16):
    from contextlib import ExitStack
    from concourse import bacc, tile, mybir
    import concourse.bass as bass
    from concourse.masks import make_identity

    f32 = mybir.dt.float32
    f16 = mybir.dt.float16
    wdt = f16 if wdt16 else f32

    nc = bacc.Bacc("TRN2", target_bir_lowering=False, debug=False,
                   num_devices=N_CORES)

    qT_ap = nc.dram_tensor("qT", [D, Q], wdt, kind="ExternalInput").ap()
    keys_ap = nc.dram_tensor("keys", [K, D], wdt, kind="ExternalInput").ap()
    keysT_ap = nc.dram_tensor("keysT", [D, K], wdt, kind="ExternalInput").ap()
    wqT_ap = nc.dram_tensor("wqT", [D, NU], wdt, kind="ExternalInput").ap()
    wkT_ap = nc.dram_tensor("wkT", [D, NU], wdt, kind="ExternalInput").ap()
    v16_ap = nc.dram_tensor("v16", [128, UT], f16, kind="ExternalInput").ap()
    biasb_ap = nc.dram_tensor("biasb", [128, UT], f32, kind="ExternalInput").ap()
    ctx_out_ap = nc.dram_tensor("ctx_out", [Q, D], f32, kind="ExternalOutput").ap()
    sn_out_ap = nc.dram_tensor("sn_out", [Q, K], f32, kind="ExternalOutput").ap()

    Tanh = mybir.ActivationFunctionType.Tanh
    Exp = mybir.ActivationFunctionType.Exp

    if variant == "io":
        # I/O-matched null: same dram tensors, minimal compute
        with tile.TileContext(nc) as tc:
            with ExitStack() as ctx:
                pool = ctx.enter_context(tc.tile_pool(name="p", bufs=2))
                t1 = pool.tile([64, D], f32)
                nc.vector.memset(t1[:, :], 0.0)
                nc.sync.dma_start(out=ctx_out_ap[:, :], in_=t1[:, :])
                nc.sync.dma_start(out=sn_out_ap[:, :], in_=t1[:, :])
        nc.compile()
        return nc

    with tile.TileContext(nc) as tc:
        with ExitStack() as ctx:
            singles = ctx.enter_context(tc.tile_pool(name="singles", bufs=1))
            work = ctx.enter_context(tc.tile_pool(name="work", bufs=1))
            s_pool = ctx.enter_context(tc.tile_pool(name="s", bufs=2))
            t_pool = ctx.enter_context(tc.tile_pool(name="t", bufs=8))
            ps_proj = ctx.enter_context(
                tc.tile_pool(name="ps_proj", bufs=1, space="PSUM"))
            ps_sc = ctx.enter_context(
                tc.tile_pool(name="ps_sc", bufs=2, space="PSUM"))
            ps_tail = ctx.enter_context(
                tc.tile_pool(name="ps_tail", bufs=2, space="PSUM"))

            # ---- input tiles (critical-path DMAs first, interleaved) --------
            sb_keysT = singles.tile([128, DT, K], wdt)
            sb_wkT = singles.tile([128, DT, NU], wdt)
            sb_qT = singles.tile([128, DT, Q], wdt)
            sb_wqT = singles.tile([128, DT, NU], wdt)
            sb_keys = singles.tile([128, KT, D], wdt)
            sb_v16 = singles.tile([128, UT], f16)
            sb_biasb = singles.tile([128, UT], f32)
            nc.gpsimd.dma_start(out=sb_qT[:, :, :],
                                in_=qT_ap.rearrange("(t p) k -> p t k", p=128))
            nc.gpsimd.dma_start(out=sb_v16[:, :], in_=v16_ap[:, :])
            nc.gpsimd.dma_start(out=sb_biasb[:, :], in_=biasb_ap[:, :])
            # wkT: first u-slice (for pk ut=0) before the rest
            nc.sync.dma_start(
                out=sb_wkT[:, :, 0:128],
                in_=wkT_ap[:, 0:128].rearrange("(t p) k -> p t k", p=128))
            for t2 in range(DT // 2):
                sl = slice(t2 * 256, (t2 + 1) * 256)
                nc.sync.dma_start(
                    out=sb_keysT[:, 2 * t2:2 * t2 + 2, :],
                    in_=keysT_ap[sl, :].rearrange("(t p) k -> p t k", p=128))
            nc.sync.dma_start(
                out=sb_wkT[:, :, 128:512],
                in_=wkT_ap[:, 128:512].rearrange("(t p) k -> p t k", p=128))
            for t2 in range(DT // 2):
                sl = slice(t2 * 256, (t2 + 1) * 256)
                nc.gpsimd.dma_start(
                    out=sb_wqT[:, 2 * t2:2 * t2 + 2, :],
                    in_=wqT_ap[sl, :].rearrange("(t p) k -> p t k", p=128))
            # only needed by the context matmul at the tail
            nc.gpsimd.dma_start(out=sb_keys[:, :, :],
                                in_=keys_ap.rearrange("(t p) k -> p t k", p=128))

            identity = singles.tile([128, 128], f32)
            make_identity(nc, identity[:, :])

            # prime the ACT table set containing both exp and tanh
            prime = singles.tile([1, 1], f32)
            nc.vector.memset(prime[:, :], 0.0)
            nc.scalar.activation(prime[:, :], prime[:, :], Exp)
            nc.scalar.activation(prime[:, :], prime[:, :], Tanh)

            do_sgen = variant not in ("nodve",)
            do_tanh = variant not in ("noact", "nodve")
            do_mm = variant not in ("nomm",)
            dummyT = None
            if not do_tanh and do_mm:
                dummyT = singles.tile([128, QBMAX, K], f16)
                nc.vector.memset(dummyT[:, :, :], 0.25)

            for _rep in range(repeat):
                # ---- projections: pkT[u,k] first (critical), then pqT -------
                pkTs, pqTs = [], []
                for ut in range(UT):
                    pk_ps = ps_proj.tile([128, K], f32, tag="pk")
                    for dt in range(DT):
                        nc.tensor.matmul(
                            out=pk_ps[:, :],
                            lhsT=sb_wkT[:, dt, ut * 128:(ut + 1) * 128],
                            rhs=sb_keysT[:, dt, :],
                            start=(dt == 0), stop=(dt == DT - 1))
                    pkT = work.tile([128, K], f32, tag=f"pkT{ut}")
                    nc.vector.tensor_copy(pkT[:, :], pk_ps[:, :])
                    pkTs.append(pkT)

                    pq_ps = ps_proj.tile([128, Q], f32, tag="pq")
                    for dt in range(DT):
                        nc.tensor.matmul(
                            out=pq_ps[:, :],
                            lhsT=sb_wqT[:, dt, ut * 128:(ut + 1) * 128],
                            rhs=sb_qT[:, dt, :],
                            start=(dt == 0), stop=(dt == DT - 1))
                    # fold normalize_bias while copying out of PSUM
                    pqT = work.tile([128, Q], f32, tag=f"pqT{ut}")
                    nc.vector.tensor_scalar_add(
                        out=pqT[:, :], in0=pq_ps[:, :],
                        scalar1=sb_biasb[:, ut:ut + 1])
                    pqTs.append(pqT)

                # ---- hot loop with per-half tail ----------------------------
                for half in range(Q // QH):
                    psum_scT = ps_sc.tile([128, KT, QH], f32, tag="scT")
                    if not do_mm:
                        nc.vector.memset(psum_scT[:, :, :], 0.001)
                    joff = 0
                    for qbsize in CHUNKS[half]:
                        q0 = half * QH + joff
                        Ts = []
                        for ut in range(UT):
                            if do_sgen:
                                S = s_pool.tile([128, QBMAX, K], f32, tag="S")
                                for j in range(qbsize):
                                    nc.vector.tensor_scalar_add(
                                        out=S[:, j, :], in0=pkTs[ut][:, :],
                                        scalar1=pqTs[ut][:, q0 + j:q0 + j + 1])
                            if do_tanh:
                                T = t_pool.tile([128, QBMAX, K], f16, tag="T")
                                nc.scalar.activation(
                                    T[:, :qbsize, :], S[:, :qbsize, :], Tanh)
                                Ts.append(T)
                            else:
                                Ts.append(dummyT)
                        if do_mm:
                            for j in range(qbsize):
                                jh = joff + j
                                for kt in range(KT):
                                    for ut in range(UT):
                                        nc.tensor.matmul(
                                            out=psum_scT[:, kt, jh:jh + 1],
                                            lhsT=Ts[ut][:, j, kt * 128:(kt + 1) * 128],
                                            rhs=sb_v16[:, ut:ut + 1],
                                            start=(ut == 0), stop=(ut == UT - 1))
                        joff += qbsize

                    # ---- tail for this q-half -------------------------------
                    q0 = half * QH
                    scT_sb = work.tile([128, KT, QH], f32, tag="scT_sb")
                    nc.vector.tensor_copy(scT_sb[:, :, :], psum_scT[:, :, :])
                    if wdt16:
                        scT16 = work.tile([128, KT, QH], f16, tag="scT16")
                        nc.vector.tensor_copy(scT16[:, :, :], psum_scT[:, :, :])
                    else:
                        scT16 = scT_sb

                    psum_sc = ps_tail.tile([QH, K], f32, tag="sc")
                    for kt in range(KT):
                        nc.tensor.transpose(
                            out=psum_sc[:, kt * 128:(kt + 1) * 128],
                            in_=scT_sb[:, kt, :], identity=identity[:, :])

                    negmax = work.tile([QH, 1], f32, tag="negmax")
                    nc.vector.tensor_reduce(
                        out=negmax[:, :], in_=psum_sc[:, :],
                        axis=mybir.AxisListType.X, op=mybir.AluOpType.max,
                        negate=True)
                    Etile = work.tile([QH, K], f32, tag="E")
                    nc.scalar.activation(Etile[:, :], psum_sc[:, :], Exp,
                                         bias=negmax[:, :])
                    ssum = work.tile([QH, 1], f32, tag="ssum")
                    nc.vector.tensor_reduce(
                        out=ssum[:, :], in_=Etile[:, :],
                        axis=mybir.AxisListType.X, op=mybir.AluOpType.add)
                    rinv = work.tile([QH, 1], f32, tag="rinv")
                    nc.vector.reciprocal(rinv[:, :], ssum[:, :])
                    SN = work.tile([QH, K], f32, tag="SN")
                    nc.vector.tensor_scalar_mul(out=SN[:, :], in0=Etile[:, :],
                                                scalar1=rinv[:, :])
                    nc.sync.dma_start(out=sn_out_ap[q0:q0 + QH, :],
                                      in_=SN[:, :])

                    psum_ctx = ps_tail.tile([QH, D], f32, tag="ctx")
                    for kt in range(KT):
                        nc.tensor.matmul(
                            out=psum_ctx[:, :],
                            lhsT=scT16[:, kt, :],
                            rhs=sb_keys[:, kt, :],
                            start=(kt == 0), stop=(kt == KT - 1))
                    ctx_sb = work.tile([QH, D], f32, tag="ctx_sb")
                    nc.vector.tensor_copy(ctx_sb[:, :], psum_ctx[:, :])
                    nc.sync.dma_start(out=ctx_out_ap[q0:q0 + QH, :],
                                      in_=ctx_sb[:, :])

    nc.compile()
    return nc


def _get_nc():
    if "nc" not in _CACHE:
        _CACHE["nc"] = _build()
    return _CACHE["nc"]


def _prep_inputs(query, keys, Wq, Wk, linear_att, normalize_scalar,
                 normalize_bias):
    query = np.asarray(query, dtype=np.float32)
    keys = np.asarray(keys, dtype=np.float32)
    Wq = np.asarray(Wq, dtype=np.float32)
    Wk = np.asarray(Wk, dtype=np.float32)
    linear_att = np.asarray(linear_att, dtype=np.float32)
    normalize_scalar = np.asarray(normalize_scalar, dtype=np.float32)
    normalize_bias = np.asarray(normalize_bias, dtype=np.float32)

    v = (linear_att / np.linalg.norm(linear_att)) * normalize_scalar[0]
    v16 = np.ascontiguousarray(v.reshape(UT, 128).T).astype(np.float16)
    biasb = np.ascontiguousarray(normalize_bias.reshape(UT, 128).T)
    wt = np.float16 if WDT16 else np.float32
    wqT = np.ascontiguousarray(Wq.T).astype(wt)
    wkT = np.ascontiguousarray(Wk.T).astype(wt)

    in_maps = []
    for b in range(B):
        in_maps.append({
            "qT": np.ascontiguousarray(query[b].T).astype(wt),
            "keys": np.ascontiguousarray(keys[b]).astype(wt),
            "keysT": np.ascontiguousarray(keys[b].T).astype(wt),
            "wqT": wqT,
            "wkT": wkT,
            "v16": v16,
            "biasb": biasb,
        })
    return in_maps


def kernel(query, keys, Wq, Wk, linear_att, normalize_scalar, normalize_bias):
    from concourse.bass_utils import run_bass_kernel_spmd

    nc = _get_nc()
    in_maps = _prep_inputs(query, keys, Wq, Wk, linear_att, normalize_scalar,
                           normalize_bias)
    res = run_bass_kernel_spmd(nc, in_maps, core_ids=list(range(N_CORES)))
    context = np.stack([res.results[b]["ctx_out"] for b in range(B)])
    scores_normalized = np.stack([res.results[b]["sn_out"] for b in range(B)])
    return context.astype(np.float32), scores_normalized.astype(np.float32)


# revision 36
# speedup vs baseline: 164.7127x; 164.7127x over previous
"""Bahdanau additive attention kernel for 8 Trainium2 NeuronCores.

Math (per batch element b):
    pq = query[b] @ Wq.T                       [Q, NU]
    pk = keys[b]  @ Wk.T                       [K, NU]
    v  = linear_att / ||linear_att|| * normalize_scalar
    scores[q,k] = sum_u tanh(pq[q,u] + pk[k,u] + bias[u]) * v[u]
    scores_normalized = softmax(scores, -1)
    context = scores @ keys[b]                 (un-normalized scores, faithful)

Sharding: data parallel over batch, B == 8 == n_cores, no collectives.

Per-core pipeline (ACT tanh over Q*K*NU = 16.7M elements is the roofline,
~110us at 128 lanes x 1.2 GHz; everything else hides under it):
    PE   : pqT[u,q], pkT[u,k] projections (fp16 matmuls, fp32 accum)
    DVE  : S[u, (q,k)-chunk] = pkT + pq[q]   (tensor_scalar add, 2x mode)
    ACT  : T = tanh(S) in large-free-dim instructions, output fp16
    PE   : scoresT[k,q] = sum_u T[u,k] * v[u]  (fp16 matvec, PSUM accum)
    per q-half tail: PE transpose + softmax + context (overlaps next half)
Chunk sizes ramp small->large->small so ACT starts ~10us in and the final
matvec burst before the tail chain is short.
"""

import sys

for _p in ("/opt/trn_rl_repo",):
    if _p not in sys.path:
        sys.path.insert(0, _p)

import numpy as np

B, Q, K, D, NU = 8, 64, 512, 512, 512
UT = NU // 128  # u tiles
KT = K // 128   # k tiles
DT = D // 128   # d tiles
QH = 32         # q's per tail half
# variable hot-loop chunk sizes per half: small at head (fast ACT ramp) and
# at the very end (small final matvec burst before the tail chain)
CHUNKS = [[2, 4, 8, 8, 10], [10, 10, 8, 2, 2]]
QBMAX = 10
N_CORES = 8
WDT16 = True    # fp16 weights/keys for projection + context matmuls

_CACHE = {}


def _build(variant="full", repeat=1, wdt16=WDT16):
    from contextlib import ExitStack
    from concourse import bacc, tile, mybir
    import concourse.bass as bass
    from concourse.masks import make_identity

    f32 = mybir.dt.float32
    f16 = mybir.dt.float16
    wdt = f16 if wdt16 else f32

    nc = bacc.Bacc("TRN2", target_bir_lowering=False, debug=False,
                   num_devices=N_CORES)

    qT_ap = nc.dram_tensor("qT", [D, Q], wdt, kind="ExternalInput").ap()
    keys_ap = nc.dram_tensor("keys", [K, D], wdt, kind="ExternalInput").ap()
    keysT_ap = nc.dram_tensor("keysT", [D, K], wdt, kind="ExternalInput").ap()
    wqT_ap = nc.dram_tensor("wqT", [D, NU], wdt, kind="ExternalInput").ap()
    wkT_ap = nc.dram_tensor("wkT", [D, NU], wdt, kind="ExternalInput").ap()
    v16_ap = nc.dram_tensor("v16", [128, UT], f16, kind="ExternalInput").ap()
    biasb_ap = nc.dram_tensor("biasb", [128, UT], f32, kind="ExternalInput").ap()
    ctx_out_ap = nc.dram_tensor("ctx_out", [Q, D], f32, kind="ExternalOutput").ap()
    sn_out_ap = nc.dram_tensor("sn_out", [Q, K], f32, kind="ExternalOutput").ap()

    Tanh = mybir.ActivationFunctionType.Tanh
    Exp = mybir.ActivationFunctionType.Exp

    if variant == "io":
        # I/O-matched null: same dram tensors, minimal compute
        with tile.TileContext(nc) as tc:
            with ExitStack() as ctx:
                pool = ctx.enter_context(tc.tile_pool(name="p", bufs=2))
                t1 = pool.tile([64, D], f32)
                nc.vector.memset(t1[:, :], 0.0)
                nc.sync.dma_start(out=ctx_out_ap[:, :], in_=t1[:, :])
                nc.sync.dma_start(out=sn_out_ap[:, :], in_=t1[:, :])
        nc.compile()
        return nc

    with tile.TileContext(nc) as tc:
        with ExitStack() as ctx:
            singles = ctx.enter_context(tc.tile_pool(name="singles", bufs=1))
            work = ctx.enter_context(tc.tile_pool(name="work", bufs=1))
            s_pool = ctx.enter_context(tc.tile_pool(name="s", bufs=2))
            t_pool = ctx.enter_context(tc.tile_pool(name="t", bufs=8))
            ps_proj = ctx.enter_context(
                tc.tile_pool(name="ps_proj", bufs=1, space="PSUM"))
            ps_sc = ctx.enter_context(
                tc.tile_pool(name="ps_sc", bufs=2, space="PSUM"))
            ps_tail = ctx.enter_context(
                tc.tile_pool(name="ps_tail", bufs=2, space="PSUM"))

            # ---- input tiles (critical-path DMAs first, interleaved) --------
            sb_keysT = singles.tile([128, DT, K], wdt)
            sb_wkT = singles.tile([128, DT, NU], wdt)
            sb_qT = singles.tile([128, DT, Q], wdt)
            sb_wqT = singles.tile([128, DT, NU], wdt)
            sb_keys = singles.tile([128, KT, D], wdt)
            sb_v16 = singles.tile([128, UT], f16)
            sb_biasb = singles.tile([128, UT], f32)
            nc.gpsimd.dma_start(out=sb_qT[:, :, :],
                                in_=qT_ap.rearrange("(t p) k -> p t k", p=128))
            nc.gpsimd.dma_start(out=sb_v16[:, :], in_=v16_ap[:, :])
            nc.gpsimd.dma_start(out=sb_biasb[:, :], in_=biasb_ap[:, :])
            # wkT: first u-slice (for pk ut=0) before the rest
            nc.sync.dma_start(
                out=sb_wkT[:, :, 0:128],
                in_=wkT_ap[:, 0:128].rearrange("(t p) k -> p t k", p=128))
            for t2 in range(DT // 2):
                sl = slice(t2 * 256, (t2 + 1) * 256)
                nc.sync.dma_start(
                    out=sb_keysT[:, 2 * t2:2 * t2 + 2, :],
                    in_=keysT_ap[sl, :].rearrange("(t p) k -> p t k", p=128))
            nc.sync.dma_start(
                out=sb_wkT[:, :, 128:512],
                in_=wkT_ap[:, 128:512].rearrange("(t p) k -> p t k", p=128))
            for t2 in range(DT // 2):
                sl = slice(t2 * 256, (t2 + 1) * 256)
                nc.gpsimd.dma_start(
                    out=sb_wqT[:, 2 * t2:2 * t2 + 2, :],
                    in_=wqT_ap[sl, :].rearrange("(t p) k -> p t k", p=128))
            # only needed by the context matmul at the tail
            nc.gpsimd.dma_start(out=sb_keys[:, :, :],
                                in_=keys_ap.rearrange("(t p) k -> p t k", p=128))

            identity = singles.tile([128, 128], f32)
            make_identity(nc, identity[:, :])

            # prime the ACT table set containing both exp and tanh
            prime = singles.tile([1, 1], f32)
            nc.vector.memset(prime[:, :], 0.0)
            nc.scalar.activation(prime[:, :], prime[:, :], Exp)
            nc.scalar.activation(prime[:, :], prime[:, :], Tanh)

            do_sgen = variant not in ("nodve",)
            do_tanh = variant not in ("noact", "nodve")
            do_mm = variant not in ("nomm",)
            dummyT = None
            if not do_tanh and do_mm:
                dummyT = singles.tile([128, QBMAX, K], f16)
                nc.vector.memset(dummyT[:, :, :], 0.25)

            for _rep in range(repeat):
                # ---- projections: pkT[u,k] first (critical), then pqT -------
                pkTs, pqTs = [], []
                for ut in range(UT):
                    pk_ps = ps_proj.tile([128, K], f32, tag="pk")
                    for dt in range(DT):
                        nc.tensor.matmul(
                            out=pk_ps[:, :],
                            lhsT=sb_wkT[:, dt, ut * 128:(ut + 1) * 128],
                            rhs=sb_keysT[:, dt, :],
                            start=(dt == 0), stop=(dt == DT - 1))
                    pkT = work.tile([128, K], f32, tag=f"pkT{ut}")
                    nc.vector.tensor_copy(pkT[:, :], pk_ps[:, :])
                    pkTs.append(pkT)

                    pq_ps = ps_proj.tile([128, Q], f32, tag="pq")
                    for dt in range(DT):
                        nc.tensor.matmul(
                            out=pq_ps[:, :],
                            lhsT=sb_wqT[:, dt, ut * 128:(ut + 1) * 128],
                            rhs=sb_qT[:, dt, :],
                            start=(dt == 0), stop=(dt == DT - 1))
                    # fold normalize_bias while copying out of PSUM
                    pqT = work.tile([128, Q], f32, tag=f"pqT{ut}")
                    nc.vector.tensor_scalar_add(
                        out=pqT[:, :], in0=pq_ps[:, :],
                        scalar1=sb_biasb[:, ut:ut + 1])
                    pqTs.append(pqT)

                # ---- hot loop with per-half tail ----------------------------
                for half in range(Q // QH):
                    psum_scT = ps_sc.tile([128, KT, QH], f32, tag="scT")
                    if not do_mm:
                        nc.vector.memset(psum_scT[:, :, :], 0.001)
                    joff = 0
                    for qbsize in CHUNKS[half]:
                        q0 = half * QH + joff
                        Ts = []
                        for ut in range(UT):
                            if do_sgen:
                                S = s_pool.tile([128, QBMAX, K], f32, tag="S")
                                for j in range(qbsize):
                                    nc.vector.tensor_scalar_add(
                                        out=S[:, j, :], in0=pkTs[ut][:, :],
                                        scalar1=pqTs[ut][:, q0 + j:q0 + j + 1])
                            if do_tanh:
                                T = t_pool.tile([128, QBMAX, K], f16, tag="T")
                                nc.scalar.activation(
                                    T[:, :qbsize, :], S[:, :qbsize, :], Tanh)
                                Ts.append(T)
                            else:
                                Ts.append(dummyT)
                        if do_mm:
                            for j in range(qbsize):
                                jh = joff + j
                                for kt in range(KT):
                                    for ut in range(UT):
                                        nc.tensor.matmul(
                                            out=psum_scT[:, kt, jh:jh + 1],
                                            lhsT=Ts[ut][:, j, kt * 128:(kt + 1) * 128],
                                            rhs=sb_v16[:, ut:ut + 1],
                                            start=(ut == 0), stop=(ut == UT - 1))
                        joff += qbsize

                    # ---- tail for this q-half -------------------------------
                    q0 = half * QH
                    scT_sb = work.tile([128, KT, QH], f32, tag="scT_sb")
                    nc.vector.tensor_copy(scT_sb[:, :, :], psum_scT[:, :, :])
                    if wdt16:
                        scT16 = work.tile([128, KT, QH], f16, tag="scT16")
                        nc.vector.tensor_copy(scT16[:, :, :], psum_scT[:, :, :])
                    else:
                        scT16 = scT_sb

                    psum_sc = ps_tail.tile([QH, K], f32, tag="sc")
                    for kt in range(KT):
                        nc.tensor.transpose(
                            out=psum_sc[:, kt * 128:(kt + 1) * 128],
                            in_=scT_sb[:, kt, :], identity=identity[:, :])

                    negmax = work.tile([QH, 1], f32, tag="negmax")
                    nc.vector.tensor_reduce(
                        out=negmax[:, :], in_=psum_sc[:, :],
                        axis=mybir.AxisListType.X, op=mybir.AluOpType.max,
                        negate=True)
                    Etile = work.tile([QH, K], f32, tag="E")
                    nc.scalar.activation(Etile[:, :], psum_sc[:, :], Exp,
                                         bias=negmax[:, :])
                    ssum = work.tile([QH, 1], f32, tag="ssum")
                    nc.vector.tensor_reduce(
                        out=ssum[:, :], in_=Etile[:, :],
                        axis=mybir.AxisListType.X, op=mybir.AluOpType.add)
                    rinv = work.tile([QH, 1], f32, tag="rinv")
                    nc.vector.reciprocal(rinv[:, :], ssum[:, :])
                    SN = work.tile([QH, K], f32, tag="SN")
                    nc.vector.tensor_scalar_mul(out=SN[:, :], in0=Etile[:, :],
                                                scalar1=rinv[:, :])
                    nc.sync.dma_start(out=sn_out_ap[q0:q0 + QH, :],
                                      in_=SN[:, :])

                    psum_ctx = ps_tail.tile([QH, D], f32, tag="ctx")
                    for kt in range(KT):
                        nc.tensor.matmul(
                            out=psum_ctx[:, :],
                            lhsT=scT16[:, kt, :],
                            rhs=sb_keys[:, kt, :],
                            start=(kt == 0), stop=(kt == KT - 1))
                    ctx_sb = work.tile([QH, D], f32, tag="ctx_sb")
                    nc.vector.tensor_copy(ctx_sb[:, :], psum_ctx[:, :])
                    nc.sync.dma_start(out=ctx_out_ap[q0:q0 + QH, :],
                                      in_=ctx_sb[:, :])

    nc.compile()
    return nc


def _get_nc():
    if "nc" not in _CACHE:
        _CACHE["nc"] = _build()
    return _CACHE["nc"]


def _prep_inputs(query, keys, Wq, Wk, linear_att, normalize_scalar,
                 normalize_bias):
    query = np.asarray(query, dtype=np.float32)
    keys = np.asarray(keys, dtype=np.float32)
    Wq = np.asarray(Wq, dtype=np.float32)
    Wk = np.asarray(Wk, dtype=np.float32)
    linear_att = np.asarray(linear_att, dtype=np.float32)
    normalize_scalar = np.asarray(normalize_scalar, dtype=np.float32)
    normalize_bias = np.asarray(normalize_bias, dtype=np.float32)

    v = (linear_att / np.linalg.norm(linear_att)) * normalize_scalar[0]
    v16 = np.ascontiguousarray(v.reshape(UT, 128).T).astype(np.float16)
    biasb = np.ascontiguousarray(normalize_bias.reshape(UT, 128).T)
    wt = np.float16 if WDT16 else np.float32
    wqT = np.ascontiguousarray(Wq.T).astype(wt)
    wkT = np.ascontiguousarray(Wk.T).astype(wt)

    in_maps = []
    for b in range(B):
        in_maps.append({
            "qT": np.ascontiguousarray(query[b].T).astype(wt),
            "keys": np.ascontiguousarray(keys[b]).astype(wt),
            "keysT": np.ascontiguousarray(keys[b].T).astype(wt),
            "wqT": wqT,
            "wkT": wkT,
            "v16": v16,
            "biasb": biasb,
        })
    return in_maps


def kernel(query, keys, Wq, Wk, linear_att, normalize_scalar, normalize_bias):
    from concourse.bass_utils import run_bass_kernel_spmd

    nc = _get_nc()
    in_maps = _prep_inputs(query, keys, Wq, Wk, linear_att, normalize_scalar,
                           normalize_bias)
    res = run_bass_kernel_spmd(nc, in_maps, core_ids=list(range(N_CORES)))
    context = np.stack([res.results[b]["ctx_out"] for b in range(B)])
    scores_normalized = np.stack([res.results[b]["sn_out"] for b in range(B)])
    return context.astype(np.float32), scores_normalized.astype(np.float32)


# revision 37
# speedup vs baseline: 167.8941x; 1.0193x over previous
"""Bahdanau additive attention kernel for 8 Trainium2 NeuronCores.

Math (per batch element b):
    pq = query[b] @ Wq.T                       [Q, NU]
    pk = keys[b]  @ Wk.T                       [K, NU]
    v  = linear_att / ||linear_att|| * normalize_scalar
    scores[q,k] = sum_u tanh(pq[q,u] + pk[k,u] + bias[u]) * v[u]
    scores_normalized = softmax(scores, -1)
    context = scores @ keys[b]                 (un-normalized scores, faithful)

Sharding: data parallel over batch, B == 8 == n_cores, no collectives.

Per-core pipeline (ACT tanh over Q*K*NU = 16.7M elements is the roofline,
~110us at 128 lanes x 1.2 GHz; everything else hides under it):
    PE   : pqT[u,q], pkT[u,k] projections (fp16 matmuls, fp32 accum)
    DVE  : S[u, (q,k)-chunk] = pkT + pq[q]   (tensor_scalar add, 2x mode)
    ACT  : T = tanh(S) in large-free-dim instructions, output fp16
    PE   : scoresT[k,q] = sum_u T[u,k] * v[u]  (fp16 matvec, PSUM accum)
    per q-half tail: PE transpose + softmax + context (overlaps next half)
Chunk sizes ramp small->large->small so ACT starts ~10us in and the final
matvec burst before the tail chain is short.
"""

import sys

for _p in ("/opt/trn_rl_repo",):
    if _p not in sys.path:
        sys.path.insert(0, _p)

import numpy as np

B, Q, K, D, NU = 8, 64, 512, 512, 512
UT = NU // 128  # u tiles
KT = K // 128   # k tiles
DT = D // 128   # d tiles
QH = 32         # q's per tail half
# variable hot-loop chunk sizes per half: small at head (fast ACT ramp) and
# at the very end (small final matvec burst before the tail chain)
CHUNKS = [[2, 4, 8, 8, 10], [10, 10, 8, 2, 2]]
QBMAX = 10
N_CORES = 8
WDT16 = True    # fp16 weights/keys for projection + context matmuls

_CACHE = {}


def _build(variant="full", repeat=1, wdt16=WDT16):
    from contextlib import ExitStack
    from concourse import bacc, tile, mybir
    import concourse.bass as bass
    from concourse.masks import make_identity

    f32 = mybir.dt.float32
    f16 = mybir.dt.float16
    wdt = f16 if wdt16 else f32

    nc = bacc.Bacc("TRN2", target_bir_lowering=False, debug=False,
                   num_devices=N_CORES)

    qT_ap = nc.dram_tensor("qT", [D, Q], wdt, kind="ExternalInput").ap()
    keys_ap = nc.dram_tensor("keys", [K, D], wdt, kind="ExternalInput").ap()
    keysT_ap = nc.dram_tensor("keysT", [D, K], wdt, kind="ExternalInput").ap()
    wqT_ap = nc.dram_tensor("wqT", [D, NU], wdt, kind="ExternalInput").ap()
    wkT_ap = nc.dram_tensor("wkT", [D, NU], wdt, kind="ExternalInput").ap()
    v16_ap = nc.dram_tensor("v16", [128, UT], f16, kind="ExternalInput").ap()
    biasb_ap = nc.dram_tensor("biasb", [128, UT], f32, kind="ExternalInput").ap()
    ctx_out_ap = nc.dram_tensor("ctx_out", [Q, D], f32, kind="ExternalOutput").ap()
    sn_out_ap = nc.dram_tensor("sn_out", [Q, K], f32, kind="ExternalOutput").ap()

    Tanh = mybir.ActivationFunctionType.Tanh
    Exp = mybir.ActivationFunctionType.Exp

    if variant == "io":
        # I/O-matched null: same dram tensors, minimal compute
        with tile.TileContext(nc) as tc:
            with ExitStack() as ctx:
                pool = ctx.enter_context(tc.tile_pool(name="p", bufs=2))
                t1 = pool.tile([64, D], f32)
                nc.vector.memset(t1[:, :], 0.0)
                nc.sync.dma_start(out=ctx_out_ap[:, :], in_=t1[:, :])
                nc.sync.dma_start(out=sn_out_ap[:, :], in_=t1[:, :])
        nc.compile()
        return nc

    with tile.TileContext(nc) as tc:
        with ExitStack() as ctx:
            singles = ctx.enter_context(tc.tile_pool(name="singles", bufs=1))
            work = ctx.enter_context(tc.tile_pool(name="work", bufs=1))
            s_pool = ctx.enter_context(tc.tile_pool(name="s", bufs=3))
            t_pool = ctx.enter_context(tc.tile_pool(name="t", bufs=8))
            ps_proj = ctx.enter_context(
                tc.tile_pool(name="ps_proj", bufs=1, space="PSUM"))
            ps_sc = ctx.enter_context(
                tc.tile_pool(name="ps_sc", bufs=2, space="PSUM"))
            ps_tail = ctx.enter_context(
                tc.tile_pool(name="ps_tail", bufs=2, space="PSUM"))

            # ---- input tiles (critical-path DMAs first, interleaved) --------
            sb_keysT = singles.tile([128, DT, K], wdt)
            sb_wkT = singles.tile([128, DT, NU], wdt)
            sb_qT = singles.tile([128, DT, Q], wdt)
            sb_wqT = singles.tile([128, DT, NU], wdt)
            sb_keys = singles.tile([128, KT, D], wdt)
            sb_v16 = singles.tile([128, UT], f16)
            sb_biasb = singles.tile([128, UT], f32)
            nc.gpsimd.dma_start(out=sb_qT[:, :, :],
                                in_=qT_ap.rearrange("(t p) k -> p t k", p=128))
            nc.gpsimd.dma_start(out=sb_v16[:, :], in_=v16_ap[:, :])
            nc.gpsimd.dma_start(out=sb_biasb[:, :], in_=biasb_ap[:, :])
            # wkT: first u-slice (for pk ut=0) before the rest
            nc.sync.dma_start(
                out=sb_wkT[:, :, 0:128],
                in_=wkT_ap[:, 0:128].rearrange("(t p) k -> p t k", p=128))
            for t2 in range(DT // 2):
                sl = slice(t2 * 256, (t2 + 1) * 256)
                nc.sync.dma_start(
                    out=sb_keysT[:, 2 * t2:2 * t2 + 2, :],
                    in_=keysT_ap[sl, :].rearrange("(t p) k -> p t k", p=128))
            nc.sync.dma_start(
                out=sb_wkT[:, :, 128:512],
                in_=wkT_ap[:, 128:512].rearrange("(t p) k -> p t k", p=128))
            for t2 in range(DT // 2):
                sl = slice(t2 * 256, (t2 + 1) * 256)
                nc.gpsimd.dma_start(
                    out=sb_wqT[:, 2 * t2:2 * t2 + 2, :],
                    in_=wqT_ap[sl, :].rearrange("(t p) k -> p t k", p=128))
            # only needed by the context matmul at the tail
            nc.gpsimd.dma_start(out=sb_keys[:, :, :],
                                in_=keys_ap.rearrange("(t p) k -> p t k", p=128))

            identity = singles.tile([128, 128], f32)
            make_identity(nc, identity[:, :])

            # prime the ACT table set containing both exp and tanh
            prime = singles.tile([1, 1], f32)
            nc.vector.memset(prime[:, :], 0.0)
            nc.scalar.activation(prime[:, :], prime[:, :], Exp)
            nc.scalar.activation(prime[:, :], prime[:, :], Tanh)

            do_sgen = variant not in ("nodve",)
            do_tanh = variant not in ("noact", "nodve")
            do_mm = variant not in ("nomm",)
            dummyT = None
            if not do_tanh and do_mm:
                dummyT = singles.tile([128, QBMAX, K], f16)
                nc.vector.memset(dummyT[:, :, :], 0.25)

            for _rep in range(repeat):
                # ---- projections: pkT[u,k] first (critical), then pqT -------
                pkTs, pqTs = [], []
                for ut in range(UT):
                    pk_ps = ps_proj.tile([128, K], f32, tag="pk")
                    for dt in range(DT):
                        nc.tensor.matmul(
                            out=pk_ps[:, :],
                            lhsT=sb_wkT[:, dt, ut * 128:(ut + 1) * 128],
                            rhs=sb_keysT[:, dt, :],
                            start=(dt == 0), stop=(dt == DT - 1))
                    pkT = work.tile([128, K], f32, tag=f"pkT{ut}")
                    nc.vector.tensor_copy(pkT[:, :], pk_ps[:, :])
                    pkTs.append(pkT)

                    pq_ps = ps_proj.tile([128, Q], f32, tag="pq")
                    for dt in range(DT):
                        nc.tensor.matmul(
                            out=pq_ps[:, :],
                            lhsT=sb_wqT[:, dt, ut * 128:(ut + 1) * 128],
                            rhs=sb_qT[:, dt, :],
                            start=(dt == 0), stop=(dt == DT - 1))
                    # fold normalize_bias while copying out of PSUM
                    pqT = work.tile([128, Q], f32, tag=f"pqT{ut}")
                    nc.vector.tensor_scalar_add(
                        out=pqT[:, :], in0=pq_ps[:, :],
                        scalar1=sb_biasb[:, ut:ut + 1])
                    pqTs.append(pqT)

                # ---- hot loop with per-half tail ----------------------------
                for half in range(Q // QH):
                    psum_scT = ps_sc.tile([128, KT, QH], f32, tag="scT")
                    if not do_mm:
                        nc.vector.memset(psum_scT[:, :, :], 0.001)
                    joff = 0
                    for qbsize in CHUNKS[half]:
                        q0 = half * QH + joff
                        Ts = []
                        for ut in range(UT):
                            if do_sgen:
                                S = s_pool.tile([128, QBMAX, K], f32, tag="S")
                                for j in range(qbsize):
                                    nc.vector.tensor_scalar_add(
                                        out=S[:, j, :], in0=pkTs[ut][:, :],
                                        scalar1=pqTs[ut][:, q0 + j:q0 + j + 1])
                            if do_tanh:
                                T = t_pool.tile([128, QBMAX, K], f16, tag="T")
                                nc.scalar.activation(
                                    T[:, :qbsize, :], S[:, :qbsize, :], Tanh)
                                Ts.append(T)
                            else:
                                Ts.append(dummyT)
                        if do_mm:
                            for j in range(qbsize):
                                jh = joff + j
                                for kt in range(KT):
                                    for ut in range(UT):
                                        nc.tensor.matmul(
                                            out=psum_scT[:, kt, jh:jh + 1],
                                            lhsT=Ts[ut][:, j, kt * 128:(kt + 1) * 128],
                                            rhs=sb_v16[:, ut:ut + 1],
                                            start=(ut == 0), stop=(ut == UT - 1))
                        joff += qbsize

                    # ---- tail for this q-half -------------------------------
                    q0 = half * QH
                    scT_sb = work.tile([128, KT, QH], f32, tag="scT_sb")
                    nc.vector.tensor_copy(scT_sb[:, :, :], psum_scT[:, :, :])
                    if wdt16:
                        scT16 = work.tile([128, KT, QH], f16, tag="scT16")
                        nc.vector.tensor_copy(scT16[:, :, :], psum_scT[:, :, :])
                    else:
                        scT16 = scT_sb

                    psum_sc = ps_tail.tile([QH, K], f32, tag="sc")
                    for kt in range(KT):
                        nc.tensor.transpose(
                            out=psum_sc[:, kt * 128:(kt + 1) * 128],
                            in_=scT_sb[:, kt, :], identity=identity[:, :])

                    negmax = work.tile([QH, 1], f32, tag="negmax")
                    nc.vector.tensor_reduce(
                        out=negmax[:, :], in_=psum_sc[:, :],
                        axis=mybir.AxisListType.X, op=mybir.AluOpType.max,
                        negate=True)
                    Etile = work.tile([QH, K], f32, tag="E")
                    ssum = work.tile([QH, 1], f32, tag="ssum")
                    nc.scalar.activation(Etile[:, :], psum_sc[:, :], Exp,
                                         bias=negmax[:, :],
                                         accum_out=ssum[:, :])
                    rinv = work.tile([QH, 1], f32, tag="rinv")
                    nc.vector.reciprocal(rinv[:, :], ssum[:, :])
                    SN = work.tile([QH, K], f32, tag="SN")
                    nc.vector.tensor_scalar_mul(out=SN[:, :], in0=Etile[:, :],
                                                scalar1=rinv[:, :])
                    nc.sync.dma_start(out=sn_out_ap[q0:q0 + QH, :],
                                      in_=SN[:, :])

                    psum_ctx = ps_tail.tile([QH, D], f32, tag="ctx")
                    for kt in range(KT):
                        nc.tensor.matmul(
                            out=psum_ctx[:, :],
                            lhsT=scT16[:, kt, :],
                            rhs=sb_keys[:, kt, :],
                            start=(kt == 0), stop=(kt == KT - 1))
                    ctx_sb = work.tile([QH, D], f32, tag="ctx_sb")
                    nc.vector.tensor_copy(ctx_sb[:, :], psum_ctx[:, :])
                    nc.sync.dma_start(out=ctx_out_ap[q0:q0 + QH, :],
                                      in_=ctx_sb[:, :])

    nc.compile()
    return nc


def _get_nc():
    if "nc" not in _CACHE:
        _CACHE["nc"] = _build()
    return _CACHE["nc"]


def _prep_inputs(query, keys, Wq, Wk, linear_att, normalize_scalar,
                 normalize_bias):
    query = np.asarray(query, dtype=np.float32)
    keys = np.asarray(keys, dtype=np.float32)
    Wq = np.asarray(Wq, dtype=np.float32)
    Wk = np.asarray(Wk, dtype=np.float32)
    linear_att = np.asarray(linear_att, dtype=np.float32)
    normalize_scalar = np.asarray(normalize_scalar, dtype=np.float32)
    normalize_bias = np.asarray(normalize_bias, dtype=np.float32)

    v = (linear_att / np.linalg.norm(linear_att)) * normalize_scalar[0]
    v16 = np.ascontiguousarray(v.reshape(UT, 128).T).astype(np.float16)
    biasb = np.ascontiguousarray(normalize_bias.reshape(UT, 128).T)
    wt = np.float16 if WDT16 else np.float32
    wqT = np.ascontiguousarray(Wq.T).astype(wt)
    wkT = np.ascontiguousarray(Wk.T).astype(wt)

    in_maps = []
    for b in range(B):
        in_maps.append({
            "qT": np.ascontiguousarray(query[b].T).astype(wt),
            "keys": np.ascontiguousarray(keys[b]).astype(wt),
            "keysT": np.ascontiguousarray(keys[b].T).astype(wt),
            "wqT": wqT,
            "wkT": wkT,
            "v16": v16,
            "biasb": biasb,
        })
    return in_maps


def kernel(query, keys, Wq, Wk, linear_att, normalize_scalar, normalize_bias):
    from concourse.bass_utils import run_bass_kernel_spmd

    nc = _get_nc()
    in_maps = _prep_inputs(query, keys, Wq, Wk, linear_att, normalize_scalar,
                           normalize_bias)
    res = run_bass_kernel_spmd(nc, in_maps, core_ids=list(range(N_CORES)))
    context = np.stack([res.results[b]["ctx_out"] for b in range(B)])
    scores_normalized = np.stack([res.results[b]["sn_out"] for b in range(B)])
    return context.astype(np.float32), scores_normalized.astype(np.float32)
